# revision 86
# baseline (speedup 1.0000x reference)
"""GQA causal attention (llama3-style RoPE) on 8 TRN2 NeuronCores.

Sharding: tensor-parallel over heads. Core c gets q-heads 4c..4c+3 and
kv-head c (GQA groups intact), plus the matching row-block of wo.T.
Each core computes a full [S, D] partial of the output projection;
the host sums the 8 partials (the "all-reduce" of the row-sharded wo).

v4: fp8e4m3 + DoubleRow matmuls where the error budget allows
(DR pairs two K=128 products per instruction at 0.5 cycles/row).
  - qkv projection: 3-chain residual split (x*32 -> hi+lo, w*1024 ->
    hi+lo; chains hi@hi + lo@hi + hi@lo) in fp8-DR: ~bf16-exact and
    25% cheaper on PE than bf16. The pre-scales keep the residuals
    inside fp8e4m3's normal range; the psum drain rescales by
    1/(32*1024). W is loaded Mtile-major so the (k|v) slice lands
    first.
  - scores (chunks j>=1): q/k quantized to fp8 after RoPE, contraction
    64 = 32 partitions x 2 DR pair slots. The [32,2,S] pair layout is
    written DIRECTLY by partition-shifted RoPE add/subs (engines allow
    out-partition != in-partition; no fold DMAs). The sin table is
    unsigned: RoPE signs live in the add/sub opcodes. Dead-zone
    zeroing via bf16 bias matmuls (-30000); diagonal causal mask is a
    gpsimd multiply on the fp8 et tiles.
  - AV (chunks j>=1): e in fp8 (exp emits fp8 directly); the DR pair
    slots carry TWO key tiles (256-key contraction, the et tile's
    native pair layout, no broadcast) with v_hi only — j>=1 attention
    is diffuse (neff ~ 300) so fp8 v noise averages out. ones-col =
    SV cancels the v-scale in the normalization.
  - chunk j=0 (rows 0..511) runs the SDPA in bf16: attention there is
    concentrated on few keys, so fp8 noise on scores/e would dominate
    the global max-error metric (which is set by the early rows).
  - wo: bf16 (y must stay >= bf16 precision); output shipped bf16,
    host sums partials in f64.
Scheduling: exp on ACT is the serial bottleneck (~76us busy); span =
j1-readiness + remaining exp + tail, so the DMA schedule prioritizes
chunk-1 x right after the chunk-0 critical set (the DMA-engine device
is a FIFO by ISSUE order — one big transfer can head-of-line-block an
urgent small one; all bulk loads are emission-interleaved prefetches).
A ~3us dummy-matmul warmup ramps the PE p-state (0.65->2.4GHz) before
the first projection. Projections are split into half-chains zipped
between scores steps; AV/norm/wo are thunks pumped between scores
pairs as PE filler, with a flush before s(3,2) and per-dc output DMAs
for the last row-chunk to shorten the tail; the final head's AV/norm
is column-split into two psum tiles so norm-half-1 and wo(sm 12,13)
start one pair before the last exp. GPSIMD must not touch
PSUM (BIR verifier), so psum drains sit on ACT/DVE; Pool carries
SBUF-only work (rope swap-muls, broadcasts, SWDGE loads).
"""

import sys

for _p in ("/opt/trn_rl_repo", "/root/.axon_site/_ro/trn_rl_repo"):
    if _p not in sys.path:
        sys.path.insert(0, _p)

import numpy as np
import ml_dtypes

import concourse.bass as bass
from concourse.alu_op_type import AluOpType
import concourse.bacc as bacc
import concourse.mybir as mybir
import concourse.tile as tile

BF16 = ml_dtypes.bfloat16
F8 = ml_dtypes.float8_e4m3
DR = mybir.MatmulPerfMode.DoubleRow

S = 2048
D = 2048
HD = 64
NH = 32
NKV = 8
NCORES = 8
QH = NH // NCORES            # 4 local q heads
QCOLS = QH * HD              # 256
MCOLS = QCOLS + 2 * HD       # 384 (q heads | k | v)
P = 128
NKP = 8                      # contraction ktile-pairs (K = 8*256)
NSQ = S // P                 # 16 seq tiles
NCH = 4
CH = 512

SX = 32.0                    # x pre-scale into fp8 normal range
SWQ = 1024.0                 # wqkv pre-scale
SV = 32.0                    # v pre-scale (cancels via ones-col = SV)
INV_PROJ = 1.0 / (SX * SWQ)
NEG = -30000.0               # causal/dead-zone bias (exp(0.125*NEG) = 0)

_CACHE = {}


def _build():
    f32 = mybir.dt.float32
    f16 = mybir.dt.float16
    bf = mybir.dt.bfloat16
    f8 = mybir.dt.float8e4

    nc = bacc.Bacc()
    xth_d = nc.dram_tensor("xth", [P, NKP, 2, S], f8, kind="ExternalInput")
    xtl_d = nc.dram_tensor("xtl", [P, NKP, 2, S], f8, kind="ExternalInput")
    # Mtile-major so the m=2 (k|v) slice loads first with full-size elems
    wth_d = nc.dram_tensor("wth", [3, P, NKP, 2, P], f8, kind="ExternalInput")
    wtl_d = nc.dram_tensor("wtl", [3, P, NKP, 2, P], f8, kind="ExternalInput")
    wot_d = nc.dram_tensor("wot", [QCOLS, D], bf, kind="ExternalInput")
    cos_d = nc.dram_tensor("cos128", [P, S], f16, kind="ExternalInput")
    swap_d = nc.dram_tensor("swap128", [P, S], f16, kind="ExternalInput")
    bias_d = nc.dram_tensor("bias512", [P, CH], bf, kind="ExternalInput")
    masks8_d = nc.dram_tensor("masks8", [P, P], f8, kind="ExternalInput")
    id128_d = nc.dram_tensor("id128", [P, P], bf, kind="ExternalInput")
    id64_d = nc.dram_tensor("id64", [HD, HD], f16, kind="ExternalInput")
    out_d = nc.dram_tensor("out", [S, D], bf, kind="ExternalOutput")

    with tile.TileContext(nc) as tc:
        with (
            tc.tile_pool(name="const", bufs=1) as cpool,
            tc.tile_pool(name="xin", bufs=1) as xpool,
            tc.tile_pool(name="big", bufs=1) as bigpool,
            tc.tile_pool(name="et", bufs=18) as epool,
            tc.tile_pool(name="et16", bufs=7) as e16pool,
            tc.tile_pool(name="tmp", bufs=3) as tpool,
            tc.tile_pool(name="otw", bufs=4) as opool,
            tc.tile_pool(name="ps_a", bufs=2, space="PSUM") as ps_a,
            tc.tile_pool(name="ps_s", bufs=2, space="PSUM") as ps_s,
            tc.tile_pool(name="ps_av", bufs=2, space="PSUM") as ps_av,
        ):
            # ---- constants / weights in ----
            cos_sb = cpool.tile([P, S], f16, tag="cos")
            swap_sb = cpool.tile([P, S], f16, tag="swap")
            bias_sb = cpool.tile([P, CH], bf, tag="bias")
            masks8_sb = cpool.tile([P, P], f8, tag="masks8")
            id128_sb = cpool.tile([P, P], bf, tag="id128")
            id64_sb = cpool.tile([HD, HD], f16, tag="id64")
            zbias = cpool.tile([P, 1], f32, tag="zbias")
            nc.gpsimd.memset(zbias[:], 0.0)

            # batched loads: one big tile per tensor, one DMA per chunk group
            # (HWDGE holds a single global device ~625ns per DMA instruction,
            # so DMA COUNT is what matters, not size)
            xh_all = xpool.tile([P, NKP, 2, S], f8, tag="xh", name="xh")
            xl_all = xpool.tile([P, NKP, 2, S], f8, tag="xl", name="xl")
            xh_sb = [xh_all[:, t] for t in range(NKP)]
            xl_sb = [xl_all[:, t] for t in range(NKP)]
            whm = [
                cpool.tile([P, NKP, 2, P], f8, tag=f"whm{m}", name=f"whm{m}")
                for m in range(3)
            ]
            wlm = [
                cpool.tile([P, NKP, 2, P], f8, tag=f"wlm{m}", name=f"wlm{m}")
                for m in range(3)
            ]
            c0 = slice(0, CH)
            c1 = slice(CH, 2 * CH)
            c23 = slice(2 * CH, S)
            # startup loads: ONLY chunk-0 data + the m=2 weight slice upfront
            # (the DMA device is a FIFO by issue order; p(2,0) is the
            # critical consumer). Later chunks are prefetched via
            # emission-interleaved issues below.
            nc.sync.dma_start(whm[2][:], wth_d[2])
            nc.sync.dma_start(xh_all[:, 0:4, :, c0], xth_d[:, 0:4, :, c0])
            nc.sync.dma_start(wlm[2][:], wtl_d[2])
            nc.sync.dma_start(xh_all[:, 4:8, :, c0], xth_d[:, 4:8, :, c0])
            nc.sync.dma_start(cos_sb[:, c0], cos_d[:, c0])
            nc.sync.dma_start(swap_sb[:, c0], swap_d[:, c0])
            nc.gpsimd.dma_start(xl_all[:, 0:4, :, c0], xtl_d[:, 0:4, :, c0])
            nc.gpsimd.dma_start(xl_all[:, 4:8, :, c0], xtl_d[:, 4:8, :, c0])
            nc.sync.dma_start(whm[0][:], wth_d[0])
            nc.sync.dma_start(wlm[0][:], wtl_d[0])
            nc.sync.dma_start(whm[1][:], wth_d[1])
            nc.sync.dma_start(wlm[1][:], wtl_d[1])
            # PE p-state warmup: ~3us of dummy matmuls on a memset tile so the
            # first real proj chain runs at full clock (PE ramps 0.65->2.4GHz
            # over ~3us of continuous execution)
            warm = cpool.tile([P, 2, P], f8, tag="warm")
            nc.vector.memset(warm[:], 0.25)
            wps = ps_a.tile([P, P], f32, tag="proj", name="ps_warm")
            with nc.named_scope("warmup"):
                for wi in range(48):
                    nc.tensor.matmul(
                        wps[:], warm[:], warm[:],
                        start=(wi == 0), stop=(wi == 47), perf_mode=DR,
                    )
            nc.gpsimd.dma_start(id64_sb[:], id64_d[:])
            nc.gpsimd.dma_start(masks8_sb[:], masks8_d[:])
            nc.gpsimd.dma_start(bias_sb[:], bias_d[:])
            nc.gpsimd.dma_start(id128_sb[:], id128_d[:])
            wot_sb = [
                cpool.tile([P, D], bf, tag=f"wot{k}", name=f"wot{k}") for k in range(2)
            ]
            crest = slice(CH, S)
            c2 = slice(2 * CH, 3 * CH)
            c3 = slice(3 * CH, S)

            # prefetch closures, fired at emission positions just ahead of
            # each consumer (keeps bulk transfers out of the fold DMAs' way)
            loads = {
                "xh_c1": lambda: nc.sync.dma_start(xh_all[:, :, :, c1], xth_d[:, :, :, c1]),
                "xl_c1": lambda: nc.gpsimd.dma_start(xl_all[:, :, :, c1], xtl_d[:, :, :, c1]),
                "cos_r": lambda: nc.sync.dma_start(cos_sb[:, crest], cos_d[:, crest]),
                "swap_r": lambda: nc.sync.dma_start(swap_sb[:, crest], swap_d[:, crest]),
                "wot0": lambda: nc.sync.dma_start(wot_sb[0][:], wot_d[0:P, :]),
                "wot1": lambda: nc.sync.dma_start(wot_sb[1][:], wot_d[P : 2 * P, :]),
                "xh_c2a": lambda: nc.sync.dma_start(xh_all[:, 0:4, :, c2], xth_d[:, 0:4, :, c2]),
                "xh_c2b": lambda: nc.sync.dma_start(xh_all[:, 4:8, :, c2], xth_d[:, 4:8, :, c2]),
                "xl_c2": lambda: nc.gpsimd.dma_start(xl_all[:, :, :, c2], xtl_d[:, :, :, c2]),
                "xh_c3a": lambda: nc.sync.dma_start(xh_all[:, 0:4, :, c3], xth_d[:, 0:4, :, c3]),
                "xh_c3b": lambda: nc.sync.dma_start(xh_all[:, 4:8, :, c3], xth_d[:, 4:8, :, c3]),
                "xl_c3": lambda: nc.gpsimd.dma_start(xl_all[:, :, :, c3], xtl_d[:, :, :, c3]),
            }

            # ---- per-head / kv tensors ----
            # fp8 pair-layout q/k for DR scores: [32, 2, S], pair dim =
            # head-dim halves (d = i*32 + p)
            qt8p = [bigpool.tile([32, 2, S], f8, tag=f"qt8p{h}", name=f"qt8p{h}") for h in range(QH)]
            kt8p = bigpool.tile([32, 2, S], f8, tag="kt8p")
            # f16 q/k for the bf16 j=0 path (chunk 0 columns only)
            qt16 = [bigpool.tile([HD, CH], f16, tag=f"qt16{h}", name=f"qt16{h}") for h in range(QH)]
            kt16 = bigpool.tile([HD, CH], f16, tag="kt16")
            vt_sb = bigpool.tile([HD, S], f16, tag="vt")
            yt_sb = [bigpool.tile([P, S], bf, tag=f"yt{m}", name=f"yt{m}") for m in range(2)]

            # [128, 2, 80] fp8 pairs: slot u = v_hi(tile 2p+u)*SV, ones=SV.
            # DR pair slots carry TWO key tiles (256-key contraction per
            # instruction); the v_lo residual is dropped — j>=1 attention is
            # diffuse (neff ~ 300) so fp8 v noise averages out.
            vaug = [None] * (NSQ // 2)
            vaugb = [None] * 4    # [128, 65] bf16 for j=0

            def vtrans(jc):
                with nc.named_scope("vtrans"):
                    for i in range(4 * jc, 4 * jc + 4):
                        pt = ps_av.tile([P, HD], f16, tag="av", name="ps_vt")
                        nc.tensor.transpose(
                            pt[:], vt_sb[:, i * P : (i + 1) * P], id64_sb[:]
                        )
                        # free-dim padded to 80: DoubleRow ldweights requires
                        # pair-slot stride % 16 == 0
                        if i % 2 == 0:
                            vaug[i // 2] = bigpool.tile(
                                [P, 2, HD + 16], f8, tag=f"vaug{i//2}", name=f"vaug{i//2}"
                            )
                        va = vaug[i // 2]
                        nc.vector.tensor_scalar_mul(va[:, i % 2, 0:HD], pt[:], SV)
                        nc.gpsimd.memset(va[:, i % 2, HD : HD + 1], SV)
                        if jc == 0:
                            vb = bigpool.tile([P, HD + 1], bf, tag=f"vaugb{i}", name=f"vaugb{i}")
                            nc.scalar.copy(vb[:, 0:HD], pt[:])
                            nc.gpsimd.memset(vb[:, HD : HD + 1], 1.0)
                            vaugb[i] = vb

            # ---- qkv projection + rope ----
            # Mtile order: kv first so SDPA can start as soon as q is ready.
            # m=2: [kT(64) | vT(64)] | m=0: q heads 0,1 | m=1: q heads 2,3
            proj_ps = {}

            def do_proj(m, j, phase=None):
                # phase 0/1: emit half the matmul chain (12 each) so PE
                # excursions between scores pairs stay ~1.3us and ACT never
                # starves; phase 1 finishes with rope+fold. phase=None: both.
                chunk = slice(j * CH, (j + 1) * CH)
                # xl-dependent chain last: its prefetch may land latest
                ops = [
                    (wt, xset, t)
                    for wt, xset in ((whm[m], xh_sb), (wlm[m], xh_sb), (whm[m], xl_sb))
                    for t in range(NKP)
                ]
                n_mm = len(ops)
                if phase == 0:
                    ps = ps_a.tile([P, CH], f32, tag="proj", name="ps_proj")
                    proj_ps[(m, j)] = ps
                    sl = range(0, n_mm // 2)
                elif phase == 1:
                    ps = proj_ps.pop((m, j))
                    sl = range(n_mm // 2, n_mm)
                else:
                    ps = ps_a.tile([P, CH], f32, tag="proj", name="ps_proj")
                    sl = range(n_mm)
                with nc.named_scope("proj"):
                    for i_mm in sl:
                        wt, xset, t = ops[i_mm]
                        nc.tensor.matmul(
                            ps[:],
                            wt[:, t],
                            xset[t][:, :, chunk],
                            start=(i_mm == 0),
                            stop=(i_mm == n_mm - 1),
                            perf_mode=DR,
                        )
                if phase == 0:
                    return
                with nc.named_scope("rope"):
                    # drain psum (scaled) to f16 staging
                    nrow = P if m < 2 else HD
                    qr = tpool.tile([P, CH], f16, tag="rope_qr", name="rope_qr")
                    nc.vector.tensor_scalar_mul(qr[:], ps[:], INV_PROJ)
                    t2 = tpool.tile([P, CH], f16, tag="rope_t2", name="rope_t2")
                    # j=0 m=2 is on the startup critical path: Pool frees the
                    # DVE serial chain there; elsewhere DVE (Pool is loaded)
                    teng = nc.gpsimd if (m < 2 or j == 0) else nc.vector
                    for rb in range(nrow // HD):
                        r0 = rb * HD
                        teng.tensor_mul(
                            t2[r0 : r0 + 32, :], qr[r0 + 32 : r0 + HD, :],
                            swap_sb[r0 + 32 : r0 + HD, chunk],
                        )
                        teng.tensor_mul(
                            t2[r0 + 32 : r0 + HD, :], qr[r0 : r0 + 32, :],
                            swap_sb[r0 : r0 + 32, chunk],
                        )
                    t3 = tpool.tile([P, CH], f16, tag="rope_t3", name="rope_t3")
                    nc.vector.tensor_mul(t3[0:nrow, :], qr[0:nrow, :], cos_sb[0:nrow, chunk])
                    # partition-shifted add/sub writes the fp8 pair layout
                    # directly (no fold DMAs — engines allow out-partition
                    # ranges that differ from the input range). The sin table
                    # is unsigned; RoPE signs live in the opcode: re = t3-t2,
                    # im = t3+t2.
                    if m < 2:
                        if j == 0:
                            # f16 staging first: s(0,h) gates on it
                            for hh in range(2):
                                h = 2 * m + hh
                                r0 = hh * HD
                                nc.vector.tensor_sub(
                                    qt16[h][0:32, :], t3[r0 : r0 + 32, :], t2[r0 : r0 + 32, :]
                                )
                                nc.vector.tensor_add(
                                    qt16[h][32:HD, :], t3[r0 + 32 : r0 + HD, :], t2[r0 + 32 : r0 + HD, :]
                                )
                        for hh in range(2):
                            h = 2 * m + hh
                            for u in range(2):
                                r0 = hh * HD + u * 32
                                eng = nc.vector if u == 0 else nc.gpsimd
                                op = eng.tensor_sub if u == 0 else eng.tensor_add
                                op(
                                    qt8p[h][:, u, chunk],
                                    t3[r0 : r0 + 32, :], t2[r0 : r0 + 32, :],
                                )
                    else:
                        if j == 0:
                            nc.vector.tensor_sub(kt16[0:32, :], t3[0:32, :], t2[0:32, :])
                            nc.vector.tensor_add(kt16[32:HD, :], t3[32:HD, :], t2[32:HD, :])
                        for u in range(2):
                            r0 = u * 32
                            eng = nc.vector if u == 0 else nc.gpsimd
                            op = eng.tensor_sub if u == 0 else eng.tensor_add
                            op(
                                kt8p[:, u, chunk],
                                t3[r0 : r0 + 32, :], t2[r0 : r0 + 32, :],
                            )
                        nc.vector.tensor_copy(vt_sb[:, chunk], qr[HD:P, :])
                        vtrans(j)

            # ---- SDPA (software-pipelined) ----
            # j=0: bf16 (concentrated attention -> fp8 noise too big)
            # j>=1: fp8 DR scores + fp8 e/v-residual AV
            # Pipeline: AV(j,h) is emitted after scores(j,h+1) so the PE
            # never waits on the exp of the head it just scored; wo of
            # chunk j-1 is emitted mid-way through chunk j's heads.
            def do_scores(j, h, pump_fn=None, inline_av_pav=None):
                nlive = 4 * j + 4
                offs = [max(0, (i - 4 * j)) * P for i in range(nlive)]
                ets = []
                with nc.named_scope("scores"):
                    for pu in range(nlive // 2):
                        if pump_fn is not None:
                            # late chunks produce filler faster than 1/pair;
                            # drain harder so the final pump-out tail is short
                            pump_fn(2 if (j == 3 and h >= 2) else 1)
                        a, b = 2 * pu, 2 * pu + 1
                        poff = offs[a]
                        ps2 = ps_s.tile([P, 2, CH], f32, tag="sc", name="ps_sc")
                        for u, i in ((0, a), (1, b)):
                            bnd = i >= nlive - 4  # boundary: needs bias
                            if j == 0:
                                nc.tensor.matmul(
                                    ps2[:, u, poff:CH],
                                    kt16[:, i * P : (i + 1) * P],
                                    qt16[h][:, poff:CH],
                                    start=True,
                                    stop=not bnd,
                                )
                            else:
                                nc.tensor.matmul(
                                    ps2[:, u, poff:CH],
                                    kt8p[:, :, i * P : (i + 1) * P],
                                    qt8p[h][:, :, j * CH + poff : (j + 1) * CH],
                                    start=True,
                                    stop=not (bnd and (offs[i] - poff) > 0),
                                    perf_mode=DR,
                                )
                            if bnd:
                                dw = offs[i] - poff
                                if j == 0 or dw > 0:
                                    nc.tensor.matmul(
                                        ps2[:, u, poff : (offs[i] + P if j == 0 else offs[i])],
                                        id128_sb[:],
                                        bias_sb[:, 384 - dw : (CH if j == 0 else 384)],
                                        start=False,
                                        stop=True,
                                    )
                        with nc.named_scope("exp"):
                            if j == 0:
                                et = e16pool.tile([P, 2, CH], bf, tag="et16", name="et16")
                            else:
                                et = epool.tile([P, 2, CH], f8, tag="et", name="et")
                            nc.scalar.activation(
                                et[:, :, poff:CH],
                                ps2[:, :, poff:CH],
                                mybir.ActivationFunctionType.Exp,
                                bias=zbias[:],
                                scale=0.125,
                            )
                        if j > 0:
                            for u, i in ((0, a), (1, b)):
                                if i >= nlive - 4:
                                    off = offs[i]
                                    with nc.named_scope("mask"):
                                        nc.gpsimd.tensor_mul(
                                            et[:, u, off : off + P],
                                            et[:, u, off : off + P],
                                            masks8_sb[:],
                                        )
                        ets.append(et)
                return ets

            # ---- filler machinery ----
            # The PE consumes scores-pairs ~2x faster than ACT can exp
            # them; since engine queues are FIFO, the PE would stall on
            # psum-buffer reuse. So AV/norm/wo work is chopped into small
            # thunks and pumped between scores pairs as PE filler.
            filler = []
            done_h = {}

            def av_thunks33(ets):
                # (3,3) is the span tail: split av by column halves into two
                # psum tiles. Half A (q-cols 0:256) completes one pair early,
                # so its norm and wo(sm 12,13) start BEFORE the last exp.
                HF = CH // 2
                pavA = ps_av.tile([HD + 1, HF], f32, tag="av", name="ps_avA")
                pavB = ps_av.tile([HD + 1, HF], f32, tag="av", name="ps_avB")

                def mk_av(pu):
                    def emit():
                        with nc.named_scope("av"):
                            if pu < 7:
                                nc.tensor.matmul(
                                    pavA[:], vaug[pu][:, :, 0 : HD + 1],
                                    ets[pu][:, :, 0:HF],
                                    start=(pu == 0), stop=(pu == 6),
                                    perf_mode=DR,
                                )
                            nc.tensor.matmul(
                                pavB[:], vaug[pu][:, :, 0 : HD + 1],
                                ets[pu][:, :, HF:CH],
                                start=(pu == 0), stop=(pu == 7),
                                perf_mode=DR,
                            )
                    return emit

                def mk_norm(pav, lo, sms):
                    def emit():
                        with nc.named_scope("norm"):
                            cs = slice(lo, lo + HF)
                            jch = slice(3 * CH + lo, 3 * CH + lo + HF)
                            recip = tpool.tile([1, CH], f32, tag="recip", name="recip")
                            bc = tpool.tile([HD, CH], f32, tag="bc", name="bc")
                            nc.vector.reciprocal(recip[:, cs], pav[HD : HD + 1, :])
                            nc.gpsimd.partition_broadcast(bc[:, cs], recip[:, cs])
                            nc.vector.tensor_mul(
                                yt_sb[1][HD:P, jch], pav[0:HD, :], bc[:, cs]
                            )
                        for sm_ in sms:
                            filler.extend(wo_thunks(sm_))
                    return emit

                return (
                    [mk_av(pu) for pu in range(7)]
                    + [mk_norm(pavA, 0, (12, 13)), mk_av(7), mk_norm(pavB, HF, (14, 15))]
                )

            def av_thunks(j, h, ets, pav_pre=None):
                nlive = 4 * j + 4
                offs = [max(0, (i - 4 * j)) * P for i in range(nlive)]
                pav = pav_pre if pav_pre is not None else ps_av.tile(
                    [HD + 1, CH], f32, tag="av", name="ps_av"
                )

                def mk_av(i):
                    # j=0: per-tile bf16; j>=1: per-PAIR fp8 DR with the et
                    # tile's native pair layout (256-key contraction/instr)
                    def emit():
                        with nc.named_scope("av"):
                            if j == 0:
                                off = offs[i]
                                nc.tensor.matmul(
                                    pav[:, off:CH],
                                    vaugb[i][:],
                                    ets[i // 2][:, i % 2, off:CH],
                                    start=(i == 0),
                                    stop=(i == nlive - 1),
                                )
                            else:
                                off = offs[2 * i]
                                nc.tensor.matmul(
                                    pav[:, off:CH],
                                    vaug[i][:, :, 0 : HD + 1],
                                    ets[i][:, :, off:CH],
                                    start=(i == 0),
                                    stop=(i == nlive // 2 - 1),
                                    perf_mode=DR,
                                )
                    return emit

                def norm():
                    with nc.named_scope("norm"):
                        qrow = (h % 2) * HD
                        recip = tpool.tile([1, CH], f32, tag="recip", name="recip")
                        bc = tpool.tile([HD, CH], f32, tag="bc", name="bc")
                        # the last norm is on the span's critical tail:
                        # pipeline it in column halves to hide stage latency
                        halves = (
                            [(0, CH // 2), (CH // 2, CH)] if (j == 3 and h == 3)
                            else [(0, CH)]
                        )
                        for lo, hi in halves:
                            cs = slice(lo, hi)
                            jch = slice(j * CH + lo, j * CH + hi)
                            nc.vector.reciprocal(recip[:, cs], pav[HD : HD + 1, cs])
                            nc.gpsimd.partition_broadcast(bc[:, cs], recip[:, cs])
                            nc.vector.tensor_mul(
                                yt_sb[h // 2][qrow : qrow + HD, jch],
                                pav[0:HD, cs], bc[:, cs],
                            )
                    done_h[j] = done_h.get(j, 0) + 1
                    if done_h[j] == QH and j != 3:
                        for sm_ in range(4 * j, 4 * j + 4):
                            filler.extend(wo_thunks(sm_))

                if pav_pre is not None:
                    return [norm]
                n_av = nlive if j == 0 else nlive // 2
                return [mk_av(i) for i in range(n_av)] + [norm]

            def wo_thunks(sm):
                srow = slice(sm * P, (sm + 1) * P)
                ot = opool.tile([P, D], bf, tag="ot", name="ot")

                def mk_dc(dcJ):
                    def emit():
                        dch = slice(dcJ * CH, (dcJ + 1) * CH)
                        pw = ps_a.tile([P, CH], f32, tag="proj", name="ps_wo")
                        with nc.named_scope("wo"):
                            for k in range(2):
                                nc.tensor.matmul(
                                    pw[:],
                                    yt_sb[k][:, srow],
                                    wot_sb[k][:, dch],
                                    start=(k == 0),
                                    stop=(k == 1),
                                )
                        with nc.named_scope("outdma"):
                            if sm >= 12 and dcJ % 2 == 0:
                                nc.scalar.copy(ot[:, dch], pw[:])
                            elif sm >= 12:
                                nc.vector.tensor_copy(ot[:, dch], pw[:])
                            else:
                                nc.vector.tensor_copy(ot[:, dch], pw[:])
                    return emit

                def mk_dma(d0, d1, eng):
                    def emit():
                        with nc.named_scope("outdma"):
                            eng.dma_start(
                                out_d[srow, d0 * CH : d1 * CH], ot[:, d0 * CH : d1 * CH]
                            )
                    return emit

                if sm >= 12:
                    # tail: ship each dc chunk as soon as it drains
                    return [
                        mk_dc(0), mk_dma(0, 1, nc.sync),
                        mk_dc(1), mk_dma(1, 2, nc.gpsimd),
                        mk_dc(2), mk_dma(2, 3, nc.sync),
                        mk_dc(3), mk_dma(3, 4, nc.gpsimd),
                    ]
                return [
                    mk_dc(0), mk_dc(1), mk_dma(0, 2, nc.sync),
                    mk_dc(2), mk_dc(3), mk_dma(2, 4, nc.gpsimd),
                ]

            def pump(n):
                done = 0
                while done < n and filler:
                    filler.pop(0)()
                    done += 1

            def step(j, h):
                ets = do_scores(j, h, pump)
                if j == 3 and h == 3:
                    filler.extend(av_thunks33(ets))
                else:
                    filler.extend(av_thunks(j, h, ets))

            # zipper emission: projections split in half-phases and spread
            # singly between scores steps so the ACT exp train never starves
            emission = [
                ("pa", 2, 0), ("pb", 2, 0),
                ("L", "cos_r", 0), ("L", "xh_c1", 0),
                ("pa", 0, 0), ("pb", 0, 0),
                ("L", "swap_r", 0), ("L", "xl_c1", 0),
                ("s", 0, 0),
                ("pa", 1, 0), ("s", 0, 1), ("pb", 1, 0),
                ("L", "wot0", 0), ("L", "xh_c2a", 0), ("L", "xl_c2", 0),
                ("s", 0, 2),
                ("pa", 2, 1), ("s", 0, 3), ("pb", 2, 1),
                ("L", "xh_c2b", 0), ("L", "wot1", 0),
                ("pa", 0, 1), ("pb", 0, 1),
                ("L", "xh_c3a", 0), ("L", "xl_c3", 0),
                ("s", 1, 0),
                ("pa", 1, 1), ("s", 1, 1), ("pb", 1, 1),
                ("L", "xh_c3b", 0),
                ("pa", 2, 2), ("s", 1, 2), ("pb", 2, 2),
                ("pa", 0, 2), ("pb", 0, 2), ("s", 1, 3),
                ("s", 2, 0),
                ("pa", 1, 2), ("s", 2, 1), ("pb", 1, 2),
                ("pa", 2, 3), ("s", 2, 2), ("pb", 2, 3),
                ("pa", 0, 3), ("pb", 0, 3), ("s", 2, 3),
                ("pa", 1, 3), ("pb", 1, 3),
                ("s", 3, 0),
                ("s", 3, 1),
                ("F", 30, 0),
                ("s", 3, 2), ("s", 3, 3),
            ]
            for kind, a, b in emission:
                if kind == "pa":
                    do_proj(a, b, 0)
                elif kind == "pb":
                    do_proj(a, b, 1)
                elif kind == "L":
                    loads[a]()
                elif kind == "F":
                    pump(a)
                else:
                    step(a, b)
            while filler:
                pump(1000)

    nc.finalize()
    return nc


def _host_inputs(x, freqs_cos, freqs_sin, wq, wk, wv, wo):
    """Build the 8 per-core input maps (all host-side preprocessing)."""
    x = np.asarray(x, np.float32)
    cos = np.asarray(freqs_cos, np.float32)  # [S, 32]
    sin = np.asarray(freqs_sin, np.float32)
    wq = np.asarray(wq, np.float32)
    wk = np.asarray(wk, np.float32)
    wv = np.asarray(wv, np.float32)
    wo = np.asarray(wo, np.float32)

    perm = np.concatenate([np.arange(0, HD, 2), np.arange(1, HD, 2)])  # de-interleave

    xt = np.ascontiguousarray(x[0].T) * SX
    xh = xt.astype(F8)
    xl = (xt - xh.astype(np.float32)).astype(F8)

    def pairs(a, ncol):
        # [P, NKP, 2, ncol]: partition-major so one DMA loads all ktiles
        return np.ascontiguousarray(
            a.reshape(NKP, 2, P, ncol).transpose(2, 0, 1, 3)
        )

    xh_p = pairs(xh, S)
    xl_p = pairs(xl, S)

    # cos128[d, t] = cos[t, d % 32]; swap128 = +sin (signs live in the
    # kernel's add/sub opcodes)
    cos128 = np.empty((P, S), np.float16)
    swap128 = np.empty((P, S), np.float16)
    for dd in range(P):
        i = dd % 32
        cos128[dd] = cos[:, i]
        swap128[dd] = sin[:, i]

    # bias512: cols 0:384 = NEG (dead zones); cols 384+c = NEG if c < p (strict upper)
    bias512 = np.full((P, CH), NEG, np.float32)
    pp = np.arange(P)[:, None]
    cc = np.arange(P)[None, :]
    bias512[:, 384:] = np.where(cc < pp, NEG, 0.0)
    bias512 = bias512.astype(BF16)

    id128 = np.eye(P, dtype=np.float32).astype(BF16)
    masks8 = (np.arange(P)[:, None] <= np.arange(P)[None, :]).astype(np.float32).astype(F8)
    id64 = np.eye(HD, dtype=np.float32).astype(np.float16)

    in_maps = []
    for c in range(NCORES):
        wq_c = wq[c * QCOLS : (c + 1) * QCOLS].reshape(QH, HD, D)[:, perm, :].reshape(
            QCOLS, D
        )
        wk_c = wk[c * HD : (c + 1) * HD][perm, :]
        wv_c = wv[c * HD : (c + 1) * HD]
        wqkvt = np.ascontiguousarray(
            np.concatenate([wq_c, wk_c, wv_c], axis=0).T
        ) * SWQ
        wh = wqkvt.astype(F8)
        wl = (wqkvt - wh.astype(np.float32)).astype(F8)

        def wpairs(a):
            # [3, P, NKP, 2, P]: Mtile-major [m][part][ktile][pairslot][col]
            return np.ascontiguousarray(
                a.reshape(NKP, 2, P, 3, P).transpose(3, 2, 0, 1, 4)
            )

        wot = np.ascontiguousarray(wo[:, c * QCOLS : (c + 1) * QCOLS].T).astype(BF16)
        in_maps.append(
            {
                "xth": xh_p,
                "xtl": xl_p,
                "wth": wpairs(wh),
                "wtl": wpairs(wl),
                "wot": wot,
                "cos128": cos128,
                "swap128": swap128,
                "bias512": bias512,
                "masks8": masks8,
                "id128": id128,
                "id64": id64,
            }
        )
    return in_maps


def kernel(x, freqs_cos, freqs_sin, wq, wk, wv, wo):
    from concourse.bass_utils import run_bass_kernel_spmd

    if "nc" not in _CACHE:
        _CACHE["nc"] = _build()
    nc = _CACHE["nc"]
    in_maps = _host_inputs(x, freqs_cos, freqs_sin, wq, wk, wv, wo)
    res = run_bass_kernel_spmd(nc, in_maps, core_ids=list(range(NCORES)))
    out = np.zeros((S, D), np.float64)
    for r in res.results:
        out += r["out"].astype(np.float64)
    return out.astype(np.float32).reshape(1, S, D)



# revision 92
# speedup vs baseline: 1.0022x; 1.0022x over previous
"""GQA causal attention (llama3-style RoPE) on 8 TRN2 NeuronCores.

Sharding: tensor-parallel over heads. Core c gets q-heads 4c..4c+3 and
kv-head c (GQA groups intact), plus the matching row-block of wo.T.
Each core computes a full [S, D] partial of the output projection;
the host sums the 8 partials (the "all-reduce" of the row-sharded wo).

v4: fp8e4m3 + DoubleRow matmuls where the error budget allows
(DR pairs two K=128 products per instruction at 0.5 cycles/row).
  - qkv projection: 3-chain residual split (x*32 -> hi+lo, w*1024 ->
    hi+lo; chains hi@hi + lo@hi + hi@lo) in fp8-DR: ~bf16-exact and
    25% cheaper on PE than bf16. The pre-scales keep the residuals
    inside fp8e4m3's normal range; the psum drain rescales by
    1/(32*1024). W is loaded Mtile-major so the (k|v) slice lands
    first.
  - scores (chunks j>=1): q/k quantized to fp8 after RoPE, contraction
    64 = 32 partitions x 2 DR pair slots. The [32,2,S] pair layout is
    written DIRECTLY by partition-shifted RoPE add/subs (engines allow
    out-partition != in-partition; no fold DMAs). The sin table is
    unsigned: RoPE signs live in the add/sub opcodes. Dead-zone
    zeroing via bf16 bias matmuls (-30000); diagonal causal mask is a
    gpsimd multiply on the fp8 et tiles.
  - AV (chunks j>=1): e in fp8 (exp emits fp8 directly); the DR pair
    slots carry TWO key tiles (256-key contraction, the et tile's
    native pair layout, no broadcast) with v_hi only — j>=1 attention
    is diffuse (neff ~ 300) so fp8 v noise averages out. ones-col =
    SV cancels the v-scale in the normalization.
  - chunk j=0 (rows 0..511) runs the SDPA in bf16: attention there is
    concentrated on few keys, so fp8 noise on scores/e would dominate
    the global max-error metric (which is set by the early rows).
  - wo: bf16 (y must stay >= bf16 precision); output shipped bf16,
    host sums partials in f64.
Scheduling: exp on ACT is the serial bottleneck (~76us busy); span =
j1-readiness + remaining exp + tail, so the DMA schedule prioritizes
chunk-1 x right after the chunk-0 critical set (the DMA-engine device
is a FIFO by ISSUE order — one big transfer can head-of-line-block an
urgent small one; all bulk loads are emission-interleaved prefetches).
A ~3us dummy-matmul warmup ramps the PE p-state (0.65->2.4GHz) before
the first projection. Projections are split into half-chains zipped
between scores steps; AV/norm/wo are thunks pumped between scores
pairs as PE filler, with a flush before s(3,2) and per-dc output DMAs
for the last row-chunk to shorten the tail; the final head's AV/norm
is column-split into two psum tiles so norm-half-1 and wo(sm 12,13)
start one pair before the last exp. GPSIMD must not touch
PSUM (BIR verifier), so psum drains sit on ACT/DVE; Pool carries
SBUF-only work (rope swap-muls, broadcasts, SWDGE loads).
"""

import sys

for _p in ("/opt/trn_rl_repo", "/root/.axon_site/_ro/trn_rl_repo"):
    if _p not in sys.path:
        sys.path.insert(0, _p)

import numpy as np
import ml_dtypes

import concourse.bass as bass
from concourse.alu_op_type import AluOpType
import concourse.bacc as bacc
import concourse.mybir as mybir
import concourse.tile as tile

BF16 = ml_dtypes.bfloat16
F8 = ml_dtypes.float8_e4m3
DR = mybir.MatmulPerfMode.DoubleRow

S = 2048
D = 2048
HD = 64
NH = 32
NKV = 8
NCORES = 8
QH = NH // NCORES            # 4 local q heads
QCOLS = QH * HD              # 256
MCOLS = QCOLS + 2 * HD       # 384 (q heads | k | v)
P = 128
NKP = 8                      # contraction ktile-pairs (K = 8*256)
NSQ = S // P                 # 16 seq tiles
NCH = 4
CH = 512

SX = 32.0                    # x pre-scale into fp8 normal range
SWQ = 1024.0                 # wqkv pre-scale
SV = 32.0                    # v pre-scale (cancels via ones-col = SV)
INV_PROJ = 1.0 / (SX * SWQ)
NEG = -30000.0               # causal/dead-zone bias (exp(0.125*NEG) = 0)

_CACHE = {}


def _build():
    f32 = mybir.dt.float32
    f16 = mybir.dt.float16
    bf = mybir.dt.bfloat16
    f8 = mybir.dt.float8e4

    nc = bacc.Bacc()
    xth_d = nc.dram_tensor("xth", [P, NKP, 2, S], f8, kind="ExternalInput")
    xtl_d = nc.dram_tensor("xtl", [P, NKP, 2, S], f8, kind="ExternalInput")
    # Mtile-major so the m=2 (k|v) slice loads first with full-size elems
    wth_d = nc.dram_tensor("wth", [3, P, NKP, 2, P], f8, kind="ExternalInput")
    wtl_d = nc.dram_tensor("wtl", [3, P, NKP, 2, P], f8, kind="ExternalInput")
    wot_d = nc.dram_tensor("wot", [QCOLS, D], bf, kind="ExternalInput")
    cos_d = nc.dram_tensor("cos128", [P, S], f16, kind="ExternalInput")
    swap_d = nc.dram_tensor("swap128", [P, S], f16, kind="ExternalInput")
    bias_d = nc.dram_tensor("bias512", [P, CH], bf, kind="ExternalInput")
    masks8_d = nc.dram_tensor("masks8", [P, P], f8, kind="ExternalInput")
    id128_d = nc.dram_tensor("id128", [P, P], bf, kind="ExternalInput")
    id64_d = nc.dram_tensor("id64", [HD, HD], f16, kind="ExternalInput")
    out_d = nc.dram_tensor("out", [S, D], bf, kind="ExternalOutput")

    with tile.TileContext(nc) as tc:
        with (
            tc.tile_pool(name="const", bufs=1) as cpool,
            tc.tile_pool(name="xin", bufs=1) as xpool,
            tc.tile_pool(name="big", bufs=1) as bigpool,
            tc.tile_pool(name="et", bufs=18) as epool,
            tc.tile_pool(name="et16", bufs=7) as e16pool,
            tc.tile_pool(name="tmp", bufs=3) as tpool,
            tc.tile_pool(name="otw", bufs=4) as opool,
            tc.tile_pool(name="ps_a", bufs=2, space="PSUM") as ps_a,
            tc.tile_pool(name="ps_s", bufs=2, space="PSUM") as ps_s,
            tc.tile_pool(name="ps_av", bufs=2, space="PSUM") as ps_av,
        ):
            # ---- constants / weights in ----
            cos_sb = cpool.tile([P, S], f16, tag="cos")
            swap_sb = cpool.tile([P, S], f16, tag="swap")
            bias_sb = cpool.tile([P, CH], bf, tag="bias")
            masks8_sb = cpool.tile([P, P], f8, tag="masks8")
            id128_sb = cpool.tile([P, P], bf, tag="id128")
            id64_sb = cpool.tile([HD, HD], f16, tag="id64")
            zbias = cpool.tile([P, 1], f32, tag="zbias")
            nc.gpsimd.memset(zbias[:], 0.0)

            # batched loads: one big tile per tensor, one DMA per chunk group
            # (HWDGE holds a single global device ~625ns per DMA instruction,
            # so DMA COUNT is what matters, not size)
            xh_all = xpool.tile([P, NKP, 2, S], f8, tag="xh", name="xh")
            xl_all = xpool.tile([P, NKP, 2, S], f8, tag="xl", name="xl")
            xh_sb = [xh_all[:, t] for t in range(NKP)]
            xl_sb = [xl_all[:, t] for t in range(NKP)]
            whm = [
                cpool.tile([P, NKP, 2, P], f8, tag=f"whm{m}", name=f"whm{m}")
                for m in range(3)
            ]
            wlm = [
                cpool.tile([P, NKP, 2, P], f8, tag=f"wlm{m}", name=f"wlm{m}")
                for m in range(3)
            ]
            c0 = slice(0, CH)
            c1 = slice(CH, 2 * CH)
            c23 = slice(2 * CH, S)
            # startup loads: ONLY chunk-0 data + the m=2 weight slice upfront
            # (the DMA device is a FIFO by issue order; p(2,0) is the
            # critical consumer). Later chunks are prefetched via
            # emission-interleaved issues below.
            nc.sync.dma_start(whm[2][:], wth_d[2])
            nc.sync.dma_start(xh_all[:, 0:4, :, c0], xth_d[:, 0:4, :, c0])
            nc.sync.dma_start(wlm[2][:], wtl_d[2])
            nc.sync.dma_start(xh_all[:, 4:8, :, c0], xth_d[:, 4:8, :, c0])
            nc.sync.dma_start(cos_sb[:, c0], cos_d[:, c0])
            nc.sync.dma_start(swap_sb[:, c0], swap_d[:, c0])
            nc.gpsimd.dma_start(xl_all[:, 0:4, :, c0], xtl_d[:, 0:4, :, c0])
            nc.gpsimd.dma_start(xl_all[:, 4:8, :, c0], xtl_d[:, 4:8, :, c0])
            nc.sync.dma_start(whm[0][:], wth_d[0])
            nc.sync.dma_start(wlm[0][:], wtl_d[0])
            nc.sync.dma_start(whm[1][:], wth_d[1])
            nc.sync.dma_start(wlm[1][:], wtl_d[1])
            # PE p-state warmup: ~3us of dummy matmuls on a memset tile so the
            # first real proj chain runs at full clock (PE ramps 0.65->2.4GHz
            # over ~3us of continuous execution)
            warm = cpool.tile([P, 2, P], f8, tag="warm")
            nc.vector.memset(warm[:], 0.25)
            wps = ps_a.tile([P, P], f32, tag="proj", name="ps_warm")
            with nc.named_scope("warmup"):
                for wi in range(48):
                    nc.tensor.matmul(
                        wps[:], warm[:], warm[:],
                        start=(wi == 0), stop=(wi == 47), perf_mode=DR,
                    )
            nc.gpsimd.dma_start(id64_sb[:], id64_d[:])
            nc.gpsimd.dma_start(masks8_sb[:], masks8_d[:])
            nc.gpsimd.dma_start(bias_sb[:], bias_d[:])
            nc.gpsimd.dma_start(id128_sb[:], id128_d[:])
            wot_sb = [
                cpool.tile([P, D], bf, tag=f"wot{k}", name=f"wot{k}") for k in range(2)
            ]
            crest = slice(CH, S)
            c2 = slice(2 * CH, 3 * CH)
            c3 = slice(3 * CH, S)

            # prefetch closures, fired at emission positions just ahead of
            # each consumer (keeps bulk transfers out of the fold DMAs' way)
            loads = {
                "xh_c1": lambda: nc.sync.dma_start(xh_all[:, :, :, c1], xth_d[:, :, :, c1]),
                "xl_c1": lambda: nc.gpsimd.dma_start(xl_all[:, :, :, c1], xtl_d[:, :, :, c1]),
                "cos_r": lambda: nc.sync.dma_start(cos_sb[:, crest], cos_d[:, crest]),
                "swap_r": lambda: nc.sync.dma_start(swap_sb[:, crest], swap_d[:, crest]),
                "wot0": lambda: nc.sync.dma_start(wot_sb[0][:], wot_d[0:P, :]),
                "wot1": lambda: nc.sync.dma_start(wot_sb[1][:], wot_d[P : 2 * P, :]),
                "xh_c2a": lambda: nc.sync.dma_start(xh_all[:, 0:4, :, c2], xth_d[:, 0:4, :, c2]),
                "xh_c2b": lambda: nc.sync.dma_start(xh_all[:, 4:8, :, c2], xth_d[:, 4:8, :, c2]),
                "xl_c2": lambda: nc.gpsimd.dma_start(xl_all[:, :, :, c2], xtl_d[:, :, :, c2]),
                "xh_c3a": lambda: nc.sync.dma_start(xh_all[:, 0:4, :, c3], xth_d[:, 0:4, :, c3]),
                "xh_c3b": lambda: nc.sync.dma_start(xh_all[:, 4:8, :, c3], xth_d[:, 4:8, :, c3]),
                "xl_c3": lambda: nc.gpsimd.dma_start(xl_all[:, :, :, c3], xtl_d[:, :, :, c3]),
            }

            # ---- per-head / kv tensors ----
            # fp8 pair-layout q/k for DR scores: [32, 2, S], pair dim =
            # head-dim halves (d = i*32 + p)
            qt8p = [bigpool.tile([32, 2, S], f8, tag=f"qt8p{h}", name=f"qt8p{h}") for h in range(QH)]
            kt8p = bigpool.tile([32, 2, S], f8, tag="kt8p")
            # f16 q/k for the bf16 j=0 path (chunk 0 columns only)
            qt16 = [bigpool.tile([HD, CH], f16, tag=f"qt16{h}", name=f"qt16{h}") for h in range(QH)]
            kt16 = bigpool.tile([HD, CH], f16, tag="kt16")
            vt_sb = bigpool.tile([HD, S], f16, tag="vt")
            yt_sb = [bigpool.tile([P, S], bf, tag=f"yt{m}", name=f"yt{m}") for m in range(2)]

            # [128, 2, 80] fp8 pairs: slot u = v_hi(tile 2p+u)*SV, ones=SV.
            # DR pair slots carry TWO key tiles (256-key contraction per
            # instruction); the v_lo residual is dropped — j>=1 attention is
            # diffuse (neff ~ 300) so fp8 v noise averages out.
            vaug = [None] * (NSQ // 2)
            vaugb = [None] * 4    # [128, 65] bf16 for j=0

            def vtrans(jc):
                with nc.named_scope("vtrans"):
                    for i in range(4 * jc, 4 * jc + 4):
                        pt = ps_av.tile([P, HD], f16, tag="av", name="ps_vt")
                        nc.tensor.transpose(
                            pt[:], vt_sb[:, i * P : (i + 1) * P], id64_sb[:]
                        )
                        # free-dim padded to 80: DoubleRow ldweights requires
                        # pair-slot stride % 16 == 0
                        if i % 2 == 0:
                            vaug[i // 2] = bigpool.tile(
                                [P, 2, HD + 16], f8, tag=f"vaug{i//2}", name=f"vaug{i//2}"
                            )
                        va = vaug[i // 2]
                        nc.vector.tensor_scalar_mul(va[:, i % 2, 0:HD], pt[:], SV)
                        nc.gpsimd.memset(va[:, i % 2, HD : HD + 1], SV)
                        if jc == 0:
                            vb = bigpool.tile([P, HD + 1], bf, tag=f"vaugb{i}", name=f"vaugb{i}")
                            nc.scalar.copy(vb[:, 0:HD], pt[:])
                            nc.gpsimd.memset(vb[:, HD : HD + 1], 1.0)
                            vaugb[i] = vb

            # ---- qkv projection + rope ----
            # Mtile order: kv first so SDPA can start as soon as q is ready.
            # m=2: [kT(64) | vT(64)] | m=0: q heads 0,1 | m=1: q heads 2,3
            proj_ps = {}

            def do_proj(m, j, phase=None):
                # phase 0/1: emit half the matmul chain (12 each) so PE
                # excursions between scores pairs stay ~1.3us and ACT never
                # starves; phase 1 finishes with rope+fold. phase=None: both.
                chunk = slice(j * CH, (j + 1) * CH)
                # xl-dependent chain last: its prefetch may land latest
                ops = [
                    (wt, xset, t)
                    for wt, xset in ((whm[m], xh_sb), (wlm[m], xh_sb), (whm[m], xl_sb))
                    for t in range(NKP)
                ]
                n_mm = len(ops)
                if phase == 0:
                    ps = ps_a.tile([P, CH], f32, tag="proj", name="ps_proj")
                    proj_ps[(m, j)] = ps
                    sl = range(0, n_mm // 2)
                elif phase == 1:
                    ps = proj_ps.pop((m, j))
                    sl = range(n_mm // 2, n_mm)
                else:
                    ps = ps_a.tile([P, CH], f32, tag="proj", name="ps_proj")
                    sl = range(n_mm)
                with nc.named_scope("proj"):
                    for i_mm in sl:
                        wt, xset, t = ops[i_mm]
                        nc.tensor.matmul(
                            ps[:],
                            wt[:, t],
                            xset[t][:, :, chunk],
                            start=(i_mm == 0),
                            stop=(i_mm == n_mm - 1),
                            perf_mode=DR,
                        )
                if phase == 0:
                    return
                with nc.named_scope("rope"):
                    # drain psum (scaled) to f16 staging
                    nrow = P if m < 2 else HD
                    qr = tpool.tile([P, CH], f16, tag="rope_qr", name="rope_qr")
                    nc.vector.tensor_scalar_mul(qr[:], ps[:], INV_PROJ)
                    t2 = tpool.tile([P, CH], f16, tag="rope_t2", name="rope_t2")
                    # j=0 m=2 is on the startup critical path: Pool frees the
                    # DVE serial chain there; elsewhere DVE (Pool is loaded)
                    teng = nc.gpsimd if (m < 2 or j == 0) else nc.vector
                    for rb in range(nrow // HD):
                        r0 = rb * HD
                        teng.tensor_mul(
                            t2[r0 : r0 + 32, :], qr[r0 + 32 : r0 + HD, :],
                            swap_sb[r0 + 32 : r0 + HD, chunk],
                        )
                        teng.tensor_mul(
                            t2[r0 + 32 : r0 + HD, :], qr[r0 : r0 + 32, :],
                            swap_sb[r0 : r0 + 32, chunk],
                        )
                    t3 = tpool.tile([P, CH], f16, tag="rope_t3", name="rope_t3")
                    nc.vector.tensor_mul(t3[0:nrow, :], qr[0:nrow, :], cos_sb[0:nrow, chunk])
                    # partition-shifted add/sub writes the fp8 pair layout
                    # directly (no fold DMAs — engines allow out-partition
                    # ranges that differ from the input range). The sin table
                    # is unsigned; RoPE signs live in the opcode: re = t3-t2,
                    # im = t3+t2.
                    if m < 2:
                        if j == 0:
                            # f16 staging first: s(0,h) gates on it
                            for hh in range(2):
                                h = 2 * m + hh
                                r0 = hh * HD
                                nc.vector.tensor_sub(
                                    qt16[h][0:32, :], t3[r0 : r0 + 32, :], t2[r0 : r0 + 32, :]
                                )
                                nc.vector.tensor_add(
                                    qt16[h][32:HD, :], t3[r0 + 32 : r0 + HD, :], t2[r0 + 32 : r0 + HD, :]
                                )
                        for hh in range(2):
                            h = 2 * m + hh
                            for u in range(2):
                                r0 = hh * HD + u * 32
                                eng = nc.vector if u == 0 else nc.gpsimd
                                op = eng.tensor_sub if u == 0 else eng.tensor_add
                                op(
                                    qt8p[h][:, u, chunk],
                                    t3[r0 : r0 + 32, :], t2[r0 : r0 + 32, :],
                                )
                    else:
                        if j == 0:
                            nc.vector.tensor_sub(kt16[0:32, :], t3[0:32, :], t2[0:32, :])
                            nc.vector.tensor_add(kt16[32:HD, :], t3[32:HD, :], t2[32:HD, :])
                        for u in range(2):
                            r0 = u * 32
                            eng = nc.vector if u == 0 else nc.gpsimd
                            op = eng.tensor_sub if u == 0 else eng.tensor_add
                            op(
                                kt8p[:, u, chunk],
                                t3[r0 : r0 + 32, :], t2[r0 : r0 + 32, :],
                            )
                        nc.vector.tensor_copy(vt_sb[:, chunk], qr[HD:P, :])
                        vtrans(j)

            # ---- SDPA (software-pipelined) ----
            # j=0: bf16 (concentrated attention -> fp8 noise too big)
            # j>=1: fp8 DR scores + fp8 e/v-residual AV
            # Pipeline: AV(j,h) is emitted after scores(j,h+1) so the PE
            # never waits on the exp of the head it just scored; wo of
            # chunk j-1 is emitted mid-way through chunk j's heads.
            def do_scores(j, h, pump_fn=None, inline_av_pav=None):
                nlive = 4 * j + 4
                offs = [max(0, (i - 4 * j)) * P for i in range(nlive)]
                ets = []
                with nc.named_scope("scores"):
                    for pu in range(nlive // 2):
                        if pump_fn is not None:
                            # late chunks produce filler faster than 1/pair;
                            # drain harder so the final pump-out tail is short
                            pump_fn(4 if (j == 3 and h == 3) else (2 if (j == 3 and h == 2) else 1))
                        a, b = 2 * pu, 2 * pu + 1
                        poff = offs[a]
                        ps2 = ps_s.tile([P, 2, CH], f32, tag="sc", name="ps_sc")
                        for u, i in ((0, a), (1, b)):
                            bnd = i >= nlive - 4  # boundary: needs bias
                            if j == 0:
                                nc.tensor.matmul(
                                    ps2[:, u, poff:CH],
                                    kt16[:, i * P : (i + 1) * P],
                                    qt16[h][:, poff:CH],
                                    start=True,
                                    stop=not bnd,
                                )
                            else:
                                nc.tensor.matmul(
                                    ps2[:, u, poff:CH],
                                    kt8p[:, :, i * P : (i + 1) * P],
                                    qt8p[h][:, :, j * CH + poff : (j + 1) * CH],
                                    start=True,
                                    stop=not (bnd and (offs[i] - poff) > 0),
                                    perf_mode=DR,
                                )
                            if bnd:
                                dw = offs[i] - poff
                                if j == 0 or dw > 0:
                                    nc.tensor.matmul(
                                        ps2[:, u, poff : (offs[i] + P if j == 0 else offs[i])],
                                        id128_sb[:],
                                        bias_sb[:, 384 - dw : (CH if j == 0 else 384)],
                                        start=False,
                                        stop=True,
                                    )
                        with nc.named_scope("exp"):
                            if j == 0:
                                et = e16pool.tile([P, 2, CH], bf, tag="et16", name="et16")
                            else:
                                et = epool.tile([P, 2, CH], f8, tag="et", name="et")
                            nc.scalar.activation(
                                et[:, :, poff:CH],
                                ps2[:, :, poff:CH],
                                mybir.ActivationFunctionType.Exp,
                                bias=zbias[:],
                                scale=0.125,
                            )
                        if j > 0:
                            for u, i in ((0, a), (1, b)):
                                if i >= nlive - 4:
                                    off = offs[i]
                                    with nc.named_scope("mask"):
                                        nc.gpsimd.tensor_mul(
                                            et[:, u, off : off + P],
                                            et[:, u, off : off + P],
                                            masks8_sb[:],
                                        )
                        ets.append(et)
                return ets

            # ---- filler machinery ----
            # The PE consumes scores-pairs ~2x faster than ACT can exp
            # them; since engine queues are FIFO, the PE would stall on
            # psum-buffer reuse. So AV/norm/wo work is chopped into small
            # thunks and pumped between scores pairs as PE filler.
            filler = []
            done_h = {}

            def av_thunks33(ets):
                # (3,3) is the span tail: split av by column halves into two
                # psum tiles. Half A (q-cols 0:256) completes one pair early,
                # so its norm and wo(sm 12,13) start BEFORE the last exp.
                HF = CH // 2
                pavA = ps_av.tile([HD + 1, HF], f32, tag="av", name="ps_avA")
                pavB = ps_av.tile([HD + 1, HF], f32, tag="av", name="ps_avB")

                def mk_av(pu):
                    def emit():
                        with nc.named_scope("av"):
                            if pu < 7:
                                nc.tensor.matmul(
                                    pavA[:], vaug[pu][:, :, 0 : HD + 1],
                                    ets[pu][:, :, 0:HF],
                                    start=(pu == 0), stop=(pu == 6),
                                    perf_mode=DR,
                                )
                            nc.tensor.matmul(
                                pavB[:], vaug[pu][:, :, 0 : HD + 1],
                                ets[pu][:, :, HF:CH],
                                start=(pu == 0), stop=(pu == 7),
                                perf_mode=DR,
                            )
                    return emit

                def mk_norm(pav, lo, sms):
                    def emit():
                        with nc.named_scope("norm"):
                            cs = slice(lo, lo + HF)
                            jch = slice(3 * CH + lo, 3 * CH + lo + HF)
                            recip = tpool.tile([1, CH], f32, tag="recip", name="recip")
                            bc = tpool.tile([HD, CH], f32, tag="bc", name="bc")
                            nc.vector.reciprocal(recip[:, cs], pav[HD : HD + 1, :])
                            nc.gpsimd.partition_broadcast(bc[:, cs], recip[:, cs])
                            nc.vector.tensor_mul(
                                yt_sb[1][HD:P, jch], pav[0:HD, :], bc[:, cs]
                            )
                        for sm_ in sms:
                            filler.extend(wo_thunks(sm_))
                    return emit

                return (
                    [mk_av(pu) for pu in range(7)]
                    + [mk_norm(pavA, 0, (12, 13)), mk_av(7), mk_norm(pavB, HF, (14, 15))]
                )

            def av_thunks(j, h, ets, pav_pre=None):
                nlive = 4 * j + 4
                offs = [max(0, (i - 4 * j)) * P for i in range(nlive)]
                pav = pav_pre if pav_pre is not None else ps_av.tile(
                    [HD + 1, CH], f32, tag="av", name="ps_av"
                )

                def mk_av(i):
                    # j=0: per-tile bf16; j>=1: per-PAIR fp8 DR with the et
                    # tile's native pair layout (256-key contraction/instr)
                    def emit():
                        with nc.named_scope("av"):
                            if j == 0:
                                off = offs[i]
                                nc.tensor.matmul(
                                    pav[:, off:CH],
                                    vaugb[i][:],
                                    ets[i // 2][:, i % 2, off:CH],
                                    start=(i == 0),
                                    stop=(i == nlive - 1),
                                )
                            else:
                                off = offs[2 * i]
                                nc.tensor.matmul(
                                    pav[:, off:CH],
                                    vaug[i][:, :, 0 : HD + 1],
                                    ets[i][:, :, off:CH],
                                    start=(i == 0),
                                    stop=(i == nlive // 2 - 1),
                                    perf_mode=DR,
                                )
                    return emit

                def norm():
                    with nc.named_scope("norm"):
                        qrow = (h % 2) * HD
                        recip = tpool.tile([1, CH], f32, tag="recip", name="recip")
                        bc = tpool.tile([HD, CH], f32, tag="bc", name="bc")
                        # the last norm is on the span's critical tail:
                        # pipeline it in column halves to hide stage latency
                        halves = (
                            [(0, CH // 2), (CH // 2, CH)] if (j == 3 and h == 3)
                            else [(0, CH)]
                        )
                        for lo, hi in halves:
                            cs = slice(lo, hi)
                            jch = slice(j * CH + lo, j * CH + hi)
                            nc.vector.reciprocal(recip[:, cs], pav[HD : HD + 1, cs])
                            nc.gpsimd.partition_broadcast(bc[:, cs], recip[:, cs])
                            nc.vector.tensor_mul(
                                yt_sb[h // 2][qrow : qrow + HD, jch],
                                pav[0:HD, cs], bc[:, cs],
                            )
                    done_h[j] = done_h.get(j, 0) + 1
                    if done_h[j] == QH and j != 3:
                        for sm_ in range(4 * j, 4 * j + 4):
                            filler.extend(wo_thunks(sm_))

                if pav_pre is not None:
                    return [norm]
                n_av = nlive if j == 0 else nlive // 2
                return [mk_av(i) for i in range(n_av)] + [norm]

            def wo_thunks(sm):
                srow = slice(sm * P, (sm + 1) * P)
                ot = opool.tile([P, D], bf, tag="ot", name="ot")

                def mk_dc(dcJ):
                    def emit():
                        dch = slice(dcJ * CH, (dcJ + 1) * CH)
                        pw = ps_a.tile([P, CH], f32, tag="proj", name="ps_wo")
                        with nc.named_scope("wo"):
                            for k in range(2):
                                nc.tensor.matmul(
                                    pw[:],
                                    yt_sb[k][:, srow],
                                    wot_sb[k][:, dch],
                                    start=(k == 0),
                                    stop=(k == 1),
                                )
                        with nc.named_scope("outdma"):
                            if sm >= 12 and dcJ % 2 == 0:
                                nc.scalar.copy(ot[:, dch], pw[:])
                            elif sm >= 12:
                                nc.vector.tensor_copy(ot[:, dch], pw[:])
                            else:
                                nc.vector.tensor_copy(ot[:, dch], pw[:])
                    return emit

                def mk_dma(d0, d1, eng):
                    def emit():
                        with nc.named_scope("outdma"):
                            eng.dma_start(
                                out_d[srow, d0 * CH : d1 * CH], ot[:, d0 * CH : d1 * CH]
                            )
                    return emit

                if sm >= 12:
                    # tail: ship each dc chunk as soon as it drains
                    return [
                        mk_dc(0), mk_dma(0, 1, nc.sync),
                        mk_dc(1), mk_dma(1, 2, nc.gpsimd),
                        mk_dc(2), mk_dma(2, 3, nc.sync),
                        mk_dc(3), mk_dma(3, 4, nc.gpsimd),
                    ]
                return [
                    mk_dc(0), mk_dc(1), mk_dma(0, 2, nc.sync),
                    mk_dc(2), mk_dc(3), mk_dma(2, 4, nc.gpsimd),
                ]

            def pump(n):
                done = 0
                while done < n and filler:
                    filler.pop(0)()
                    done += 1

            def step(j, h):
                ets = do_scores(j, h, pump)
                if j == 3 and h == 3:
                    filler.extend(av_thunks33(ets))
                else:
                    filler.extend(av_thunks(j, h, ets))

            # zipper emission: projections split in half-phases and spread
            # singly between scores steps so the ACT exp train never starves
            emission = [
                ("pa", 2, 0), ("pb", 2, 0),
                ("L", "cos_r", 0), ("L", "xh_c1", 0),
                ("pa", 0, 0), ("pb", 0, 0),
                ("L", "swap_r", 0), ("L", "xl_c1", 0),
                ("s", 0, 0),
                ("pa", 1, 0), ("s", 0, 1), ("pb", 1, 0),
                ("L", "wot0", 0), ("L", "xh_c2a", 0), ("L", "xl_c2", 0),
                ("s", 0, 2),
                ("pa", 2, 1), ("s", 0, 3), ("pb", 2, 1),
                ("L", "xh_c2b", 0), ("L", "wot1", 0),
                ("pa", 0, 1), ("pb", 0, 1),
                ("L", "xh_c3a", 0), ("L", "xl_c3", 0),
                ("s", 1, 0),
                ("pa", 1, 1), ("s", 1, 1), ("pb", 1, 1),
                ("L", "xh_c3b", 0),
                ("pa", 2, 2), ("s", 1, 2), ("pb", 2, 2),
                ("pa", 0, 2), ("pb", 0, 2), ("s", 1, 3),
                ("s", 2, 0),
                ("pa", 1, 2), ("s", 2, 1), ("pb", 1, 2),
                ("pa", 2, 3), ("s", 2, 2), ("pb", 2, 3),
                ("pa", 0, 3), ("pb", 0, 3), ("s", 2, 3),
                ("pa", 1, 3), ("pb", 1, 3),
                ("s", 3, 0), ("F", 99, 0),
                ("s", 3, 1), ("F", 99, 0),
                ("F", 99, 0),
                ("s", 3, 2), ("F", 99, 0), ("s", 3, 3),
            ]
            for kind, a, b in emission:
                if kind == "pa":
                    do_proj(a, b, 0)
                elif kind == "pb":
                    do_proj(a, b, 1)
                elif kind == "L":
                    loads[a]()
                elif kind == "F":
                    pump(a)
                else:
                    step(a, b)
            while filler:
                pump(1000)

    nc.finalize()
    return nc


def _host_inputs(x, freqs_cos, freqs_sin, wq, wk, wv, wo):
    """Build the 8 per-core input maps (all host-side preprocessing)."""
    x = np.asarray(x, np.float32)
    cos = np.asarray(freqs_cos, np.float32)  # [S, 32]
    sin = np.asarray(freqs_sin, np.float32)
    wq = np.asarray(wq, np.float32)
    wk = np.asarray(wk, np.float32)
    wv = np.asarray(wv, np.float32)
    wo = np.asarray(wo, np.float32)

    perm = np.concatenate([np.arange(0, HD, 2), np.arange(1, HD, 2)])  # de-interleave

    xt = np.ascontiguousarray(x[0].T) * SX
    xh = xt.astype(F8)
    xl = (xt - xh.astype(np.float32)).astype(F8)

    def pairs(a, ncol):
        # [P, NKP, 2, ncol]: partition-major so one DMA loads all ktiles
        return np.ascontiguousarray(
            a.reshape(NKP, 2, P, ncol).transpose(2, 0, 1, 3)
        )

    xh_p = pairs(xh, S)
    xl_p = pairs(xl, S)

    # cos128[d, t] = cos[t, d % 32]; swap128 = +sin (signs live in the
    # kernel's add/sub opcodes)
    cos128 = np.empty((P, S), np.float16)
    swap128 = np.empty((P, S), np.float16)
    for dd in range(P):
        i = dd % 32
        cos128[dd] = cos[:, i]
        swap128[dd] = sin[:, i]

    # bias512: cols 0:384 = NEG (dead zones); cols 384+c = NEG if c < p (strict upper)
    bias512 = np.full((P, CH), NEG, np.float32)
    pp = np.arange(P)[:, None]
    cc = np.arange(P)[None, :]
    bias512[:, 384:] = np.where(cc < pp, NEG, 0.0)
    bias512 = bias512.astype(BF16)

    id128 = np.eye(P, dtype=np.float32).astype(BF16)
    masks8 = (np.arange(P)[:, None] <= np.arange(P)[None, :]).astype(np.float32).astype(F8)
    id64 = np.eye(HD, dtype=np.float32).astype(np.float16)

    in_maps = []
    for c in range(NCORES):
        wq_c = wq[c * QCOLS : (c + 1) * QCOLS].reshape(QH, HD, D)[:, perm, :].reshape(
            QCOLS, D
        )
        wk_c = wk[c * HD : (c + 1) * HD][perm, :]
        wv_c = wv[c * HD : (c + 1) * HD]
        wqkvt = np.ascontiguousarray(
            np.concatenate([wq_c, wk_c, wv_c], axis=0).T
        ) * SWQ
        wh = wqkvt.astype(F8)
        wl = (wqkvt - wh.astype(np.float32)).astype(F8)

        def wpairs(a):
            # [3, P, NKP, 2, P]: Mtile-major [m][part][ktile][pairslot][col]
            return np.ascontiguousarray(
                a.reshape(NKP, 2, P, 3, P).transpose(3, 2, 0, 1, 4)
            )

        wot = np.ascontiguousarray(wo[:, c * QCOLS : (c + 1) * QCOLS].T).astype(BF16)
        in_maps.append(
            {
                "xth": xh_p,
                "xtl": xl_p,
                "wth": wpairs(wh),
                "wtl": wpairs(wl),
                "wot": wot,
                "cos128": cos128,
                "swap128": swap128,
                "bias512": bias512,
                "masks8": masks8,
                "id128": id128,
                "id64": id64,
            }
        )
    return in_maps


def kernel(x, freqs_cos, freqs_sin, wq, wk, wv, wo):
    from concourse.bass_utils import run_bass_kernel_spmd

    if "nc" not in _CACHE:
        _CACHE["nc"] = _build()
    nc = _CACHE["nc"]
    in_maps = _host_inputs(x, freqs_cos, freqs_sin, wq, wk, wv, wo)
    res = run_bass_kernel_spmd(nc, in_maps, core_ids=list(range(NCORES)))
    out = np.zeros((S, D), np.float64)
    for r in res.results:
        out += r["out"].astype(np.float64)
    return out.astype(np.float32).reshape(1, S, D)

